# revision 15
# baseline (speedup 1.0000x reference)
"""Trainium2 Bass kernel for nn_DistillSTU (LDS scan + spectral contraction).

Math: out[t,d] = sum_{delta>=0} k[delta,d] * u[t-delta,d],  u = x @ M_inputs,
      k[delta,d] = sum_j W[j,d]*Bm[j]*A[j]^delta (+ dvg[d] at delta=0),
      W = (C[:,:24]+C[:,24:]) @ M_filters, dvg = (Dv[:24]+Dv[24:]) @ M_filters.

Sharding: 768 channels split across 8 cores (96 each); embarrassingly parallel.

Per-core decomposition over T=2048 (chunks L=128, subs l=8):
  base   same-sub pairs (lag 0..7): exact; PE matmuls with per-channel
         diagonal weight matrices, accumulated into per-chunk PSUM tiles.
  sub    same-chunk earlier-sub pairs: reduced-pole (r=16) state matmuls;
         (sub,pole) stacked on partitions so one matmul per 128-row block.
  chunk  earlier-chunk pairs: exact 100 poles; chunk states via one
         tensor_tensor_scan (the LDS recurrence instruction), then one
         carry matmul per chunk.
"""
import sys
import numpy as np

sys.path.insert(0, "/opt/trn_rl_repo")

T = 2048
D = 768
NJ = 100          # exact state dim
L = 128           # chunk length
NCH = T // L      # 16 chunks
SUB = 8           # sub length
NS = L // SUB     # 16 subs per chunk
R = 16            # reduced poles for sub-carries
NCORE = 8
DP = D // NCORE   # 96 channels per core

_CACHE = {}

# column offsets inside the packed constant block (partition dim = 128)
_CONST_WIDTHS = [
    ("mi", 6 * DP), ("qt", NJ), ("pt", L), ("gate", DP * NCH),
    ("wrep", DP * NCH), ("rt", (NS - 1) * R), ("p2a", 8 * SUB),
    ("p2b", 7 * SUB), ("vra", DP), ("vrb", DP), ("ktd", SUB * DP),
    ("ident", DP), ("zeros", DP),
]
CONST_OFF = {}
_off = 0
for _n, _w in _CONST_WIDTHS:
    CONST_OFF[_n] = _off
    _off += _w
CW = _off


def _derive_tables(A, Bm, C, Dv, M_filters, M_inputs):
    """All host-side parameter preprocessing (small tensors only)."""
    f8 = np.float64
    A = A.astype(f8); Bm = Bm.astype(f8)
    C = C.astype(f8); Dv = Dv.astype(f8); Mf = M_filters.astype(f8)
    W = (C[:, :24] + C[:, 24:]) @ Mf                    # (100, 768)
    dvg = (Dv[:24] + Dv[24:]) @ Mf                      # (768,)
    V100 = W * Bm[:, None]                              # (100, 768)

    # exact short kernel (lags 0..7) -> per-core diagonal weight blocks
    pows = A[None, :] ** np.arange(SUB)[:, None]        # (8, 100)
    ktab8 = pows @ V100                                 # (8, 768)
    ktab8[0] += dvg

    # reduced-pole fit of k[delta,d] on delta in [1, L-1]
    deltas = np.arange(1, L)
    kwin = (A[None, :] ** deltas[:, None]) @ V100       # (127, 768)
    lam = np.geomspace(0.008, 3.5, R)
    mu = np.exp(-lam)
    G = mu[None, :] ** deltas[:, None]                  # (127, R)
    Vr, *_ = np.linalg.lstsq(G, kwin, rcond=None)       # (R, 768)

    # chunk-level tables (exact poles)
    qt = (Bm[None, :] * A[None, :] ** (L - 1 - np.arange(L))[:, None])  # (128,100)
    pt = A[None, :] ** (np.arange(L) + 1)[:, None]      # (128, 100) -> use (100,128)
    pt = np.ascontiguousarray(pt.T)                     # pt[j, i] = A_j^{i+1}
    gate = np.zeros((NJ, DP * NCH))
    gate[:, :] = (A ** L)[:, None]
    gate[:, 0::NCH] = 0.0                               # reset at c==0 per channel

    # sub-level tables (reduced poles); col order (s,p): s=1..15, p=0..R-1
    rt = np.zeros((L, (NS - 1) * R))
    for s in range(1, NS):
        m = np.arange(SUB * s)
        rt[: SUB * s, (s - 1) * R:s * R] = mu[None, :] ** (SUB * s - 1 - m)[:, None]
    # block-diagonal carry weights: p2[(s,p), (s,r)] = mu_p^{r+1}
    p2a = np.zeros((8 * R, 8 * SUB))                    # s = 1..8
    p2b = np.zeros((7 * R, 7 * SUB))                    # s = 9..15
    pr = mu[:, None] ** (np.arange(SUB) + 1)[None, :]   # (R, 8)
    for s in range(8):
        p2a[s * R:(s + 1) * R, s * SUB:(s + 1) * SUB] = pr
    for s in range(7):
        p2b[s * R:(s + 1) * R, s * SUB:(s + 1) * SUB] = pr

    f4 = np.float32
    per_core = []
    for i in range(NCORE):
        sl = slice(i * DP, (i + 1) * DP)
        ktd = np.zeros((DP, SUB * DP), dtype=f4)        # diag blocks per lag
        for dlt in range(SUB):
            ktd[np.arange(DP), dlt * DP + np.arange(DP)] = ktab8[dlt, sl]
        wv = W[:, sl]                                    # (100, 96)
        wrep = np.repeat(wv[:, :, None], NCH, axis=2).reshape(NJ, DP * NCH)
        vra = np.zeros((8 * R, DP), dtype=f4)
        vrb = np.zeros((7 * R, DP), dtype=f4)
        for s in range(8):
            vra[s * R:(s + 1) * R] = Vr[:, sl]
        for s in range(7):
            vrb[s * R:(s + 1) * R] = Vr[:, sl]
        mi = np.ascontiguousarray(M_inputs.astype(f8)[:, sl]).astype(f4)
        # pack everything into one (128, CW) constant block -> single DMA
        blocks = {}
        cb = np.zeros((128, CW), dtype=f4)
        for name, arr in (
            ("mi", mi.reshape(6, 128, DP).transpose(1, 0, 2).reshape(128, 6 * DP)),
            ("qt", qt.astype(f4)), ("pt", pt.astype(f4)),
            ("gate", gate.astype(f4)), ("wrep", wrep.astype(f4)),
            ("rt", rt.astype(f4)), ("p2a", p2a.astype(f4)),
            ("p2b", p2b.astype(f4)), ("vra", vra), ("vrb", vrb),
            ("ktd", ktd), ("ident", np.eye(DP, dtype=f4)),
            ("zeros", np.zeros((NJ, DP), dtype=f4)),
        ):
            r0, c0 = arr.shape[0], CONST_OFF[name]
            cb[:r0, c0:c0 + arr.shape[1]] = arr
        per_core.append(dict(consts=cb))
    return per_core


def _build_nc():
    from concourse import bass, bacc, mybir, tile

    nc = bacc.Bacc()
    f4 = mybir.dt.float32
    # --- DRAM parameters
    xT = nc.declare_dram_parameter("xT", [D, T], f4, isOutput=False)
    cdram = nc.declare_dram_parameter("consts", [128, CW], f4, isOutput=False)
    out = nc.declare_dram_parameter("out", [DP, T], f4, isOutput=True)

    KT = D // L  # 6 k-tiles of 128 for the projection contraction

    def copy_op(idx, dst, src):
        if idx % 2 == 0:
            nc.scalar.copy(dst, src)
        else:
            nc.vector.tensor_copy(dst, src)

    with tile.TileContext(nc) as tc:
        with (
            tc.tile_pool(name="consts", bufs=1) as consts,
            tc.tile_pool(name="xt", bufs=1) as xtp,
            tc.tile_pool(name="work", bufs=1) as work,
            tc.tile_pool(name="f2", bufs=4) as f2p,
            tc.tile_pool(name="ps", bufs=3, space="PSUM") as psp,
            tc.tile_pool(name="carry", bufs=4, space="PSUM") as cpool,
        ):
            # ---- constant loads (single DMA for the packed block)
            call = consts.tile([128, CW], f4, tag="call")
            nc.sync.dma_start(call[:], cdram[:])

            def cs(name, rows, width, woff=0):
                c0 = CONST_OFF[name] + woff
                return call[0:rows, c0:c0 + width]

            mi_sb = [cs("mi", 128, DP, k * DP) for k in range(KT)]
            qt_sb = cs("qt", L, NJ)
            pt_sb = cs("pt", NJ, L)
            gate_sb = cs("gate", NJ, DP * NCH)
            wrep_sb = cs("wrep", NJ, DP * NCH)
            rt_sb = cs("rt", L, (NS - 1) * R)
            p2a_sb = cs("p2a", 8 * R, 8 * SUB)
            p2b_sb = cs("p2b", 7 * R, 7 * SUB)
            vra_sb = cs("vra", 8 * R, DP)
            vrb_sb = cs("vrb", 7 * R, DP)
            ktd_sb = cs("ktd", DP, SUB * DP)
            id_sb = cs("ident", DP, DP)
            zeros_sb = cs("zeros", NJ, DP)

            xt_sb = []
            for k in range(KT):
                t = xtp.tile([L, T], f4, tag=f"xt{k}", name=f"xt_sb{k}")
                nc.sync.dma_start(t[:], xT[k * L:(k + 1) * L, :])
                xt_sb.append(t)

            u_dt = work.tile([DP, T], f4, tag="u_dt")
            u_tp = work.tile([L, DP * NCH], f4, tag="u_tp")
            s_all = work.tile([NJ, DP * NCH], f4, tag="s_all")
            e_all = work.tile([NJ, DP * NCH], f4, tag="e_all")
            f_all = work.tile([NJ, DP * NCH], f4, tag="f_all")
            out_sb = work.tile([DP, T], f4, tag="out_sb")

            NSPL = T // 512  # 4 projection column splits
            # ---- projection: u_dt[d, t] = sum_e mi[e, d] * xT[e, t]
            for n in range(NSPL):
                pu = psp.tile([DP, 512], f4, tag="ps")
                for k in range(KT):
                    nc.tensor.matmul(
                        pu[:], mi_sb[k], xt_sb[k][:, n * 512:(n + 1) * 512],
                        start=(k == 0), stop=(k == KT - 1))
                nc.scalar.copy(u_dt[:, n * 512:(n + 1) * 512], pu[:])

            # ---- u_tp: per-chunk transpose of u_dt -> (time, chan)
            for c in range(NCH):
                ptp = psp.tile([L, DP], f4, tag="ps")
                nc.tensor.transpose(ptp[:], u_dt[:, c * L:(c + 1) * L], id_sb)
                nc.scalar.copy(u_tp[:, c * DP:(c + 1) * DP], ptp[:])

            # ---- chunk states S_c then one scan for cross-chunk recurrence
            for c in range(NCH):
                ps = psp.tile([NJ, DP], f4, tag="ps")
                nc.tensor.matmul(ps[:], qt_sb, u_tp[:, c * DP:(c + 1) * DP],
                                 start=True, stop=True)
                nc.vector.tensor_copy(s_all[:, c::NCH], ps[:])
            nc.vector.tensor_tensor_scan(
                e_all[:], gate_sb, s_all[:], 0.0,
                op0=mybir.AluOpType.mult, op1=mybir.AluOpType.add)
            nc.vector.tensor_tensor(f_all[:], e_all[:], wrep_sb,
                                    op=mybir.AluOpType.mult)

            # ---- per chunk: base (permuted layout) + sub/chunk carries, merge
            for c in range(NCH):
                # base triangle in PERMUTED column order: col = pos*NS + sb,
                # so each lag writes one contiguous PSUM column range.
                accp = cpool.tile([DP, L], f4, tag="car")
                for dlt in range(SUB):
                    rsrc = u_dt[:, c * L:(c + 1) * L].rearrange(
                        "d (sb l) -> d l sb", l=SUB)[:, 0:SUB - dlt, :]
                    nc.tensor.matmul(
                        accp[:, dlt * NS:L],
                        ktd_sb[:, dlt * DP:(dlt + 1) * DP], rsrc,
                        start=(dlt == 0), stop=(dlt == SUB - 1))
                # sub carries: states at sub boundaries (reduced poles)
                e2a = psp.tile([8 * R, DP], f4, tag="ps")
                nc.tensor.matmul(e2a[:], rt_sb[:, 0:8 * R],
                                 u_tp[:, c * DP:(c + 1) * DP],
                                 start=True, stop=True)
                e2b = psp.tile([7 * R, DP], f4, tag="ps")
                nc.tensor.matmul(e2b[:], rt_sb[:, 8 * R:(NS - 1) * R],
                                 u_tp[:, c * DP:(c + 1) * DP],
                                 start=True, stop=True)
                f2a = f2p.tile([8 * R, DP], f4, tag="f2a")
                nc.vector.tensor_tensor(f2a[:], e2a[:], vra_sb,
                                        op=mybir.AluOpType.mult)
                f2b = f2p.tile([7 * R, DP], f4, tag="f2b")
                nc.vector.tensor_tensor(f2b[:], e2b[:], vrb_sb,
                                        op=mybir.AluOpType.mult)
                # natural-order carry tile: chunk carry covers all 128 cols,
                # sub carries cover cols 8..127
                sacc = cpool.tile([DP, L], f4, tag="car")
                first = c == 0
                clhs = zeros_sb if first else f_all[:, (c - 1)::NCH]
                nc.tensor.matmul(sacc[:], clhs, pt_sb,
                                 start=True, stop=False)
                nc.tensor.matmul(sacc[:, SUB:SUB + 8 * SUB], f2a[:], p2a_sb,
                                 start=False, stop=False)
                nc.tensor.matmul(sacc[:, SUB + 8 * SUB:L], f2b[:], p2b_sb,
                                 start=False, stop=True)
                # merge: copy permuted base to SBUF, add strided views
                bsb = f2p.tile([DP, L], f4, tag="bsb")
                nc.vector.tensor_copy(bsb[:], accp[:])
                bview = bsb[:].rearrange("d (l sb) -> d sb l", sb=NS)
                nc.vector.tensor_tensor(
                    out_sb[:, c * L + SUB:(c + 1) * L].rearrange(
                        "d (sb l) -> d sb l", l=SUB),
                    sacc[:, SUB:L].rearrange("d (sb l) -> d sb l", l=SUB),
                    bview[:, 1:NS, :], op=mybir.AluOpType.add)
                nc.vector.tensor_tensor(
                    out_sb[:, c * L:c * L + SUB], sacc[:, 0:SUB],
                    bview[:, 0, :], op=mybir.AluOpType.add)

            nc.sync.dma_start(out[:], out_sb[:])
    nc.compile()
    return nc


def _get_program():
    if "nc" not in _CACHE:
        _CACHE["nc"] = _build_nc()
    return _CACHE["nc"]


def kernel(x, input_pos, M_inputs, M_filters, A, Bm, C, Dv, _trace=False,
           _trace_kwargs=None):
    from concourse.bass_utils import run_bass_kernel_spmd

    x = np.asarray(x, dtype=np.float32)
    per_core = _derive_tables(
        np.asarray(A), np.asarray(Bm), np.asarray(C), np.asarray(Dv),
        np.asarray(M_filters), np.asarray(M_inputs))
    xTm = np.ascontiguousarray(x[0].T)                   # (768, 2048)

    nc = _get_program()
    in_maps = [dict(xT=xTm, **per_core[i]) for i in range(NCORE)]
    kw = dict(_trace_kwargs or {})
    res = run_bass_kernel_spmd(nc, in_maps, list(range(NCORE)),
                               trace=_trace, **kw)
    _CACHE["last_result"] = res
    full = np.concatenate([res.results[i]["out"] for i in range(NCORE)], axis=0)
    return np.ascontiguousarray(full.T)[None].astype(np.float32)


if __name__ == "__main__":
    # smoke test with random inputs
    rng = np.random.default_rng(0)
    ins = dict(
        x=rng.standard_normal((1, T, D), dtype=np.float32),
        input_pos=np.arange(T, dtype=np.int32),
        M_inputs=(rng.standard_normal((D, D)) * 0.02).astype(np.float32),
        M_filters=(rng.standard_normal((24, D)) * 0.02).astype(np.float32),
        A=rng.uniform(0, 0.99, 100).astype(np.float32),
        Bm=(rng.standard_normal(100) * 0.1).astype(np.float32),
        C=(rng.standard_normal((100, 48)) * 0.1).astype(np.float32),
        Dv=(rng.standard_normal(48) * 0.1).astype(np.float32),
    )
    got = kernel(**ins)
    print("kernel output", got.shape, got.dtype, float(np.abs(got).max()))


# revision 17
# speedup vs baseline: 1.1187x; 1.1187x over previous
"""Trainium2 Bass kernel for nn_DistillSTU (LDS scan + spectral contraction).

Math: out[t,d] = sum_{delta>=0} k[delta,d] * u[t-delta,d],  u = x @ M_inputs,
      k[delta,d] = sum_j W[j,d]*Bm[j]*A[j]^delta (+ dvg[d] at delta=0),
      W = (C[:,:24]+C[:,24:]) @ M_filters, dvg = (Dv[:24]+Dv[24:]) @ M_filters.

Sharding: 768 channels split across 8 cores (96 each); embarrassingly parallel.

Per-core decomposition over T=2048 (chunks L=128, subs l=8):
  base   same-sub pairs (lag 0..7): exact; PE matmuls with per-channel
         diagonal weight matrices, accumulated into per-chunk PSUM tiles.
  sub    same-chunk earlier-sub pairs: reduced-pole (r=16) state matmuls;
         (sub,pole) stacked on partitions so one matmul per 128-row block.
  chunk  earlier-chunk pairs: exact 100 poles; chunk states via one
         tensor_tensor_scan (the LDS recurrence instruction), then one
         carry matmul per chunk.
"""
import sys
import numpy as np

sys.path.insert(0, "/opt/trn_rl_repo")

T = 2048
D = 768
NJ = 100          # exact state dim
L = 128           # chunk length
NCH = T // L      # 16 chunks
SUB = 8           # sub length
NS = L // SUB     # 16 subs per chunk
R = 16            # reduced poles for sub-carries
NCORE = 8
DP = D // NCORE   # 96 channels per core

_CACHE = {}

# column offsets inside the packed constant block (partition dim = 128)
_CONST_WIDTHS = [
    ("mi", 6 * DP), ("qt", NJ), ("pt", L), ("gate", DP * NCH),
    ("wrep", DP * NCH), ("rt", (NS - 1) * R), ("p2a", 8 * SUB),
    ("p2b", 7 * SUB), ("vra", DP), ("vrb", DP), ("ktd", SUB * DP),
    ("ident", DP), ("zeros", DP),
]
CONST_OFF = {}
_off = 0
for _n, _w in _CONST_WIDTHS:
    CONST_OFF[_n] = _off
    _off += _w
CW = _off


def _derive_tables(A, Bm, C, Dv, M_filters, M_inputs):
    """All host-side parameter preprocessing (small tensors only)."""
    f8 = np.float64
    A = A.astype(f8); Bm = Bm.astype(f8)
    C = C.astype(f8); Dv = Dv.astype(f8); Mf = M_filters.astype(f8)
    W = (C[:, :24] + C[:, 24:]) @ Mf                    # (100, 768)
    dvg = (Dv[:24] + Dv[24:]) @ Mf                      # (768,)
    V100 = W * Bm[:, None]                              # (100, 768)

    # exact short kernel (lags 0..7) -> per-core diagonal weight blocks
    pows = A[None, :] ** np.arange(SUB)[:, None]        # (8, 100)
    ktab8 = pows @ V100                                 # (8, 768)
    ktab8[0] += dvg

    # reduced-pole fit of k[delta,d] on delta in [1, L-1]
    deltas = np.arange(1, L)
    kwin = (A[None, :] ** deltas[:, None]) @ V100       # (127, 768)
    lam = np.geomspace(0.008, 3.5, R)
    mu = np.exp(-lam)
    G = mu[None, :] ** deltas[:, None]                  # (127, R)
    Vr, *_ = np.linalg.lstsq(G, kwin, rcond=None)       # (R, 768)

    # chunk-level tables (exact poles)
    qt = (Bm[None, :] * A[None, :] ** (L - 1 - np.arange(L))[:, None])  # (128,100)
    pt = A[None, :] ** (np.arange(L) + 1)[:, None]      # (128, 100) -> use (100,128)
    pt = np.ascontiguousarray(pt.T)                     # pt[j, i] = A_j^{i+1}
    gate = np.zeros((NJ, DP * NCH))
    gate[:, :] = (A ** L)[:, None]
    gate[:, 0::NCH] = 0.0                               # reset at c==0 per channel

    # sub-level tables (reduced poles); col order (s,p): s=1..15, p=0..R-1
    rt = np.zeros((L, (NS - 1) * R))
    for s in range(1, NS):
        m = np.arange(SUB * s)
        rt[: SUB * s, (s - 1) * R:s * R] = mu[None, :] ** (SUB * s - 1 - m)[:, None]
    # block-diagonal carry weights: p2[(s,p), (s,r)] = mu_p^{r+1}
    p2a = np.zeros((8 * R, 8 * SUB))                    # s = 1..8
    p2b = np.zeros((7 * R, 7 * SUB))                    # s = 9..15
    pr = mu[:, None] ** (np.arange(SUB) + 1)[None, :]   # (R, 8)
    for s in range(8):
        p2a[s * R:(s + 1) * R, s * SUB:(s + 1) * SUB] = pr
    for s in range(7):
        p2b[s * R:(s + 1) * R, s * SUB:(s + 1) * SUB] = pr

    f4 = np.float32
    per_core = []
    for i in range(NCORE):
        sl = slice(i * DP, (i + 1) * DP)
        ktd = np.zeros((DP, SUB * DP), dtype=f4)        # diag blocks per lag
        for dlt in range(SUB):
            ktd[np.arange(DP), dlt * DP + np.arange(DP)] = ktab8[dlt, sl]
        wv = W[:, sl]                                    # (100, 96)
        wrep = np.repeat(wv[:, :, None], NCH, axis=2).reshape(NJ, DP * NCH)
        vra = np.zeros((8 * R, DP), dtype=f4)
        vrb = np.zeros((7 * R, DP), dtype=f4)
        for s in range(8):
            vra[s * R:(s + 1) * R] = Vr[:, sl]
        for s in range(7):
            vrb[s * R:(s + 1) * R] = Vr[:, sl]
        mi = np.ascontiguousarray(M_inputs.astype(f8)[:, sl]).astype(f4)
        # pack everything into one (128, CW) constant block -> single DMA
        blocks = {}
        cb = np.zeros((128, CW), dtype=f4)
        for name, arr in (
            ("mi", mi.reshape(6, 128, DP).transpose(1, 0, 2).reshape(128, 6 * DP)),
            ("qt", qt.astype(f4)), ("pt", pt.astype(f4)),
            ("gate", gate.astype(f4)), ("wrep", wrep.astype(f4)),
            ("rt", rt.astype(f4)), ("p2a", p2a.astype(f4)),
            ("p2b", p2b.astype(f4)), ("vra", vra), ("vrb", vrb),
            ("ktd", ktd), ("ident", np.eye(DP, dtype=f4)),
            ("zeros", np.zeros((NJ, DP), dtype=f4)),
        ):
            r0, c0 = arr.shape[0], CONST_OFF[name]
            cb[:r0, c0:c0 + arr.shape[1]] = arr
        per_core.append(dict(consts=cb))
    return per_core


def _build_nc():
    from concourse import bass, bacc, mybir, tile

    nc = bacc.Bacc()
    f4 = mybir.dt.float32
    # --- DRAM parameters
    xT = nc.declare_dram_parameter("xT", [D, T], f4, isOutput=False)
    cdram = nc.declare_dram_parameter("consts", [128, CW], f4, isOutput=False)
    out = nc.declare_dram_parameter("out", [DP, T], f4, isOutput=True)

    KT = D // L  # 6 k-tiles of 128 for the projection contraction


    def _bview(bv, c4):
        # (96, 15, 8): sb = 1..15 within chunk c4, pos = 0..7 (stride 64)
        v = bv.rearrange("d (l sbg) -> d l sbg", l=SUB)
        return v[:, :, c4 * NS + 1:(c4 + 1) * NS].rearrange(
            "d l sb -> d sb l")

    def _bview0(bv, c4):
        # (96, 8): sb = 0 of chunk c4, pos = 0..7 (stride 64)
        v = bv.rearrange("d (l sbg) -> d l sbg", l=SUB)
        return v[:, :, c4 * NS]

    def copy_op(idx, dst, src):
        if idx % 2 == 0:
            nc.scalar.copy(dst, src)
        else:
            nc.vector.tensor_copy(dst, src)

    with tile.TileContext(nc) as tc:
        with (
            tc.tile_pool(name="consts", bufs=1) as consts,
            tc.tile_pool(name="xt", bufs=1) as xtp,
            tc.tile_pool(name="work", bufs=1) as work,
            tc.tile_pool(name="f2", bufs=4) as f2p,
            tc.tile_pool(name="ps", bufs=3, space="PSUM") as psp,
            tc.tile_pool(name="carry", bufs=3, space="PSUM") as cpool,
        ):
            # ---- constant loads (single DMA for the packed block)
            call = consts.tile([128, CW], f4, tag="call")
            nc.sync.dma_start(call[:], cdram[:])

            def cs(name, rows, width, woff=0):
                c0 = CONST_OFF[name] + woff
                return call[0:rows, c0:c0 + width]

            mi_sb = [cs("mi", 128, DP, k * DP) for k in range(KT)]
            qt_sb = cs("qt", L, NJ)
            pt_sb = cs("pt", NJ, L)
            gate_sb = cs("gate", NJ, DP * NCH)
            wrep_sb = cs("wrep", NJ, DP * NCH)
            rt_sb = cs("rt", L, (NS - 1) * R)
            p2a_sb = cs("p2a", 8 * R, 8 * SUB)
            p2b_sb = cs("p2b", 7 * R, 7 * SUB)
            vra_sb = cs("vra", 8 * R, DP)
            vrb_sb = cs("vrb", 7 * R, DP)
            ktd_sb = cs("ktd", DP, SUB * DP)
            id_sb = cs("ident", DP, DP)
            zeros_sb = cs("zeros", NJ, DP)

            xt_sb = []
            for k in range(KT):
                t = xtp.tile([L, T], f4, tag=f"xt{k}", name=f"xt_sb{k}")
                nc.sync.dma_start(t[:], xT[k * L:(k + 1) * L, :])
                xt_sb.append(t)

            u_dt = work.tile([DP, T], f4, tag="u_dt")
            u_tp = work.tile([L, DP * NCH], f4, tag="u_tp")
            s_all = work.tile([NJ, DP * NCH], f4, tag="s_all")
            e_all = work.tile([NJ, DP * NCH], f4, tag="e_all")
            f_all = work.tile([NJ, DP * NCH], f4, tag="f_all")
            out_sb = work.tile([DP, T], f4, tag="out_sb")

            NSPL = T // 512  # 4 projection column splits
            # ---- projection: u_dt[d, t] = sum_e mi[e, d] * xT[e, t]
            for n in range(NSPL):
                pu = psp.tile([DP, 512], f4, tag="ps")
                for k in range(KT):
                    nc.tensor.matmul(
                        pu[:], mi_sb[k], xt_sb[k][:, n * 512:(n + 1) * 512],
                        start=(k == 0), stop=(k == KT - 1))
                nc.scalar.copy(u_dt[:, n * 512:(n + 1) * 512], pu[:])

            # ---- u_tp: per-chunk transpose of u_dt -> (time, chan)
            for c in range(NCH):
                ptp = psp.tile([L, DP], f4, tag="ps")
                nc.tensor.transpose(ptp[:], u_dt[:, c * L:(c + 1) * L], id_sb)
                nc.scalar.copy(u_tp[:, c * DP:(c + 1) * DP], ptp[:])

            # ---- chunk states S_c then one scan for cross-chunk recurrence
            for c in range(NCH):
                ps = psp.tile([NJ, DP], f4, tag="ps")
                nc.tensor.matmul(ps[:], qt_sb, u_tp[:, c * DP:(c + 1) * DP],
                                 start=True, stop=True)
                nc.vector.tensor_copy(s_all[:, c::NCH], ps[:])
            nc.vector.tensor_tensor_scan(
                e_all[:], gate_sb, s_all[:], 0.0,
                op0=mybir.AluOpType.mult, op1=mybir.AluOpType.add)
            nc.vector.tensor_tensor(f_all[:], e_all[:], wrep_sb,
                                    op=mybir.AluOpType.mult)

            # ---- base triangle at 4-chunk-group granularity (PERMUTED
            # column order within each 512-col group: col = pos*64 + subidx)
            GL = 4 * L            # 512 time steps per group
            NG = T // GL          # 4 groups
            NSG = GL // SUB       # 64 subs per group
            bsb_g = []
            for g in range(NG):
                accp = cpool.tile([DP, GL], f4, tag="accp", name=f"accp{g}", bufs=2)
                for dlt in range(SUB):
                    rsrc = u_dt[:, g * GL:(g + 1) * GL].rearrange(
                        "d (sb l) -> d l sb", l=SUB)[:, 0:SUB - dlt, :]
                    nc.tensor.matmul(
                        accp[:, dlt * NSG:GL],
                        ktd_sb[:, dlt * DP:(dlt + 1) * DP], rsrc,
                        start=(dlt == 0), stop=(dlt == SUB - 1))
                bsb = f2p.tile([DP, GL], f4, tag="bsb", name=f"bsb{g}")
                nc.vector.tensor_copy(bsb[:], accp[:])
                bsb_g.append(bsb)

            # ---- per chunk: sub/chunk carries + merge
            for c in range(NCH):
                # sub carries: states at sub boundaries (reduced poles)
                e2a = psp.tile([8 * R, DP], f4, tag="ps")
                nc.tensor.matmul(e2a[:], rt_sb[:, 0:8 * R],
                                 u_tp[:, c * DP:(c + 1) * DP],
                                 start=True, stop=True)
                e2b = psp.tile([7 * R, DP], f4, tag="ps")
                nc.tensor.matmul(e2b[:], rt_sb[:, 8 * R:(NS - 1) * R],
                                 u_tp[:, c * DP:(c + 1) * DP],
                                 start=True, stop=True)
                f2a = f2p.tile([8 * R, DP], f4, tag="f2a")
                nc.vector.tensor_tensor(f2a[:], e2a[:], vra_sb,
                                        op=mybir.AluOpType.mult)
                f2b = f2p.tile([7 * R, DP], f4, tag="f2b")
                nc.vector.tensor_tensor(f2b[:], e2b[:], vrb_sb,
                                        op=mybir.AluOpType.mult)
                # natural-order carry tile: chunk carry covers all 128 cols,
                # sub carries cover cols 8..127
                sacc = cpool.tile([DP, L], f4, tag="sacc", bufs=3)
                first = c == 0
                clhs = zeros_sb if first else f_all[:, (c - 1)::NCH]
                nc.tensor.matmul(sacc[:], clhs, pt_sb,
                                 start=True, stop=False)
                nc.tensor.matmul(sacc[:, SUB:SUB + 8 * SUB], f2a[:], p2a_sb,
                                 start=False, stop=False)
                nc.tensor.matmul(sacc[:, SUB + 8 * SUB:L], f2b[:], p2b_sb,
                                 start=False, stop=True)
                # merge with the group-level permuted base
                g, c4 = divmod(c, 4)
                bv = bsb_g[g][:]
                nc.vector.tensor_tensor(
                    out_sb[:, c * L + SUB:(c + 1) * L].rearrange(
                        "d (sb l) -> d sb l", l=SUB),
                    sacc[:, SUB:L].rearrange("d (sb l) -> d sb l", l=SUB),
                    _bview(bv, c4), op=mybir.AluOpType.add)
                nc.vector.tensor_tensor(
                    out_sb[:, c * L:c * L + SUB], sacc[:, 0:SUB],
                    _bview0(bv, c4), op=mybir.AluOpType.add)

            nc.sync.dma_start(out[:], out_sb[:])
    nc.compile()
    return nc


def _get_program():
    if "nc" not in _CACHE:
        _CACHE["nc"] = _build_nc()
    return _CACHE["nc"]


def kernel(x, input_pos, M_inputs, M_filters, A, Bm, C, Dv, _trace=False,
           _trace_kwargs=None):
    from concourse.bass_utils import run_bass_kernel_spmd

    x = np.asarray(x, dtype=np.float32)
    per_core = _derive_tables(
        np.asarray(A), np.asarray(Bm), np.asarray(C), np.asarray(Dv),
        np.asarray(M_filters), np.asarray(M_inputs))
    xTm = np.ascontiguousarray(x[0].T)                   # (768, 2048)

    nc = _get_program()
    in_maps = [dict(xT=xTm, **per_core[i]) for i in range(NCORE)]
    kw = dict(_trace_kwargs or {})
    res = run_bass_kernel_spmd(nc, in_maps, list(range(NCORE)),
                               trace=_trace, **kw)
    _CACHE["last_result"] = res
    full = np.concatenate([res.results[i]["out"] for i in range(NCORE)], axis=0)
    return np.ascontiguousarray(full.T)[None].astype(np.float32)


if __name__ == "__main__":
    # smoke test with random inputs
    rng = np.random.default_rng(0)
    ins = dict(
        x=rng.standard_normal((1, T, D), dtype=np.float32),
        input_pos=np.arange(T, dtype=np.int32),
        M_inputs=(rng.standard_normal((D, D)) * 0.02).astype(np.float32),
        M_filters=(rng.standard_normal((24, D)) * 0.02).astype(np.float32),
        A=rng.uniform(0, 0.99, 100).astype(np.float32),
        Bm=(rng.standard_normal(100) * 0.1).astype(np.float32),
        C=(rng.standard_normal((100, 48)) * 0.1).astype(np.float32),
        Dv=(rng.standard_normal(48) * 0.1).astype(np.float32),
    )
    got = kernel(**ins)
    print("kernel output", got.shape, got.dtype, float(np.abs(got).max()))


# revision 20
# speedup vs baseline: 1.4723x; 1.3160x over previous
"""Trainium2 Bass kernel for nn_DistillSTU (LDS scan + spectral contraction).

Math: out[t,d] = sum_{delta>=0} k[delta,d] * u[t-delta,d],  u = x @ M_inputs,
      k[delta,d] = sum_j W[j,d]*Bm[j]*A[j]^delta (+ dvg[d] at delta=0),
      W = (C[:,:24]+C[:,24:]) @ M_filters, dvg = (Dv[:24]+Dv[24:]) @ M_filters.

Sharding: 768 channels split across 8 cores (96 each); embarrassingly parallel.

Per-core decomposition over T=2048 (chunks L=128, subs l=8):
  base   same-sub pairs (lag 0..7): exact short kernel; shift-FMA on
         ScalarE (lag 0) + GpSimd/VectorE (lags 1..7), d-partition layout.
  sub    same-chunk earlier-sub pairs: reduced-pole (r=8) states, batched
         across all chunks into 3 wide matmuls; one carry matmul per chunk.
  chunk  earlier-chunk pairs: exact 100 poles; chunk states via 3 wide
         matmuls + one tensor_tensor_scan; one carry matmul per chunk.
All state tensors use the (d,c)-interleaved free layout (col = d*NCH + c)
so per-chunk slices are stride-NCH column views.
"""
import sys
import numpy as np

sys.path.insert(0, "/opt/trn_rl_repo")

T = 2048
D = 768
NJ = 100          # exact state dim (chunk path)
L = 128           # chunk length
NCH = T // L      # 16 chunks
SUB = 8           # sub length
NS = L // SUB     # 16 subs per chunk
R = 8             # reduced poles for sub-carries; (s,p) = 15*8 = 120 <= 128
NCORE = 8
DP = D // NCORE   # 96 channels per core
FC = DP * NCH     # 1536 free cols of the (d,c) layout

_CACHE = {}

# column offsets inside the packed constant block (partition dim = 128)
_CONST_WIDTHS = [
    ("mi", 6 * DP), ("qt", NJ), ("pt", L), ("gate", FC),
    ("wrep", FC), ("rt", (NS - 1) * R), ("p2", (NS - 1) * SUB),
    ("vrep", FC), ("ktab", SUB), ("ident", DP), ("zeros", DP),
]
CONST_OFF = {}
_off = 0
for _n, _w in _CONST_WIDTHS:
    CONST_OFF[_n] = _off
    _off += _w
CW = _off


def _derive_tables(A, Bm, C, Dv, M_filters, M_inputs):
    """All host-side parameter preprocessing (small tensors only)."""
    f8 = np.float64
    A = A.astype(f8); Bm = Bm.astype(f8)
    C = C.astype(f8); Dv = Dv.astype(f8); Mf = M_filters.astype(f8)
    W = (C[:, :24] + C[:, 24:]) @ Mf                    # (100, 768)
    dvg = (Dv[:24] + Dv[24:]) @ Mf                      # (768,)
    V100 = W * Bm[:, None]                              # (100, 768)

    # exact short kernel (lags 0..7)
    pows = A[None, :] ** np.arange(SUB)[:, None]        # (8, 100)
    ktab8 = pows @ V100                                 # (8, 768)
    ktab8[0] += dvg

    # reduced-pole fit of k[delta,d] on delta in [1, L-1]; pole decay
    # rates refined by Nelder-Mead on the least-squares residual
    deltas = np.arange(1, L)
    kwin = (A[None, :] ** deltas[:, None]) @ V100       # (127, 768)

    def _fit(lam):
        mu = np.exp(-np.abs(lam))
        G = mu[None, :] ** deltas[:, None]
        Vr, *_ = np.linalg.lstsq(G, kwin, rcond=None)
        return mu, G, Vr, np.linalg.norm(G @ Vr - kwin)

    lam = np.geomspace(0.02, 1.5, R)
    mu, G, Vr, r0 = _fit(lam)
    try:
        from scipy.optimize import minimize
        res = minimize(lambda x: _fit(x)[3], lam, method="Nelder-Mead",
                       options={"maxiter": 3000, "fatol": 1e-12})
        mu2, G2, Vr2, r2 = _fit(res.x)
        if r2 < r0:
            mu, G, Vr = mu2, G2, Vr2
    except Exception:
        pass

    # chunk-level tables (exact poles)
    qt = (Bm[None, :] * A[None, :] ** (L - 1 - np.arange(L))[:, None])  # (128,100)
    pt = np.ascontiguousarray(
        (A[None, :] ** (np.arange(L) + 1)[:, None]).T)  # (100,128): A_j^{i+1}
    gate = np.broadcast_to((A ** L)[:, None], (NJ, FC)).copy()
    gate[:, 0::NCH] = 0.0                               # reset at c==0 per channel

    # sub-level tables (reduced poles); (s,p) order: s=1..15 outer, p inner
    rt = np.zeros((L, (NS - 1) * R))
    for s in range(1, NS):
        m = np.arange(SUB * s)
        rt[: SUB * s, (s - 1) * R:s * R] = mu[None, :] ** (SUB * s - 1 - m)[:, None]
    p2 = np.zeros(((NS - 1) * R, (NS - 1) * SUB))       # block-diag carries
    pr = mu[:, None] ** (np.arange(SUB) + 1)[None, :]   # (R, 8)
    for s in range(NS - 1):
        p2[s * R:(s + 1) * R, s * SUB:(s + 1) * SUB] = pr

    f4 = np.float32
    per_core = []
    for i in range(NCORE):
        sl = slice(i * DP, (i + 1) * DP)
        wrep = np.repeat(W[:, sl][:, :, None], NCH, axis=2).reshape(NJ, FC)
        vrep = np.zeros(((NS - 1) * R, FC))
        vr_dc = np.repeat(Vr[:, sl][:, :, None], NCH, axis=2).reshape(R, FC)
        for s in range(NS - 1):
            vrep[s * R:(s + 1) * R] = vr_dc
        mi = np.ascontiguousarray(M_inputs.astype(f8)[:, sl]).astype(f4)
        ktabT = np.ascontiguousarray(ktab8[:, sl].T)    # (96, 8)
        cb = np.zeros((128, CW), dtype=f4)
        for name, arr in (
            ("mi", mi.reshape(6, 128, DP).transpose(1, 0, 2).reshape(128, 6 * DP)),
            ("qt", qt), ("pt", pt), ("gate", gate), ("wrep", wrep),
            ("rt", rt), ("p2", p2), ("vrep", vrep), ("ktab", ktabT),
            ("ident", np.eye(DP)), ("zeros", np.zeros((NJ, DP))),
        ):
            c0 = CONST_OFF[name]
            cb[:arr.shape[0], c0:c0 + arr.shape[1]] = arr
        per_core.append(dict(consts=cb))
    return per_core


def _build_nc():
    from concourse import bass, bacc, mybir, tile

    nc = bacc.Bacc()
    f4 = mybir.dt.float32
    xT = nc.declare_dram_parameter("xT", [D, T], f4, isOutput=False)
    cdram = nc.declare_dram_parameter("consts", [128, CW], f4, isOutput=False)
    out = nc.declare_dram_parameter("out", [DP, T], f4, isOutput=True)

    KT = D // L   # 6 k-tiles for the projection contraction
    NSPL = T // 512

    with tile.TileContext(nc) as tc:
        with (
            tc.tile_pool(name="consts", bufs=1) as consts,
            tc.tile_pool(name="xt", bufs=1) as xtp,
            tc.tile_pool(name="work", bufs=1) as work,
            tc.tile_pool(name="ps", bufs=2, space="PSUM") as psp,
            tc.tile_pool(name="big", bufs=1, space="PSUM") as bigp,
            tc.tile_pool(name="carry", bufs=3, space="PSUM") as cpool,
        ):
            call = consts.tile([128, CW], f4, tag="call")
            nc.sync.dma_start(call[:], cdram[:])

            def cs(name, rows, width, woff=0):
                c0 = CONST_OFF[name] + woff
                return call[0:rows, c0:c0 + width]

            mi_sb = [cs("mi", 128, DP, k * DP) for k in range(KT)]
            qt_sb = cs("qt", L, NJ)
            pt_sb = cs("pt", NJ, L)
            gate_sb = cs("gate", NJ, FC)
            wrep_sb = cs("wrep", NJ, FC)
            rt_sb = cs("rt", L, (NS - 1) * R)
            p2_sb = cs("p2", (NS - 1) * R, (NS - 1) * SUB)
            vrep_sb = cs("vrep", (NS - 1) * R, FC)
            ktab_sb = cs("ktab", DP, SUB)
            id_sb = cs("ident", DP, DP)
            zeros_sb = cs("zeros", NJ, DP)

            xt_sb = []
            for k in range(KT):
                t = xtp.tile([L, T], f4, tag=f"xt{k}", name=f"xt_sb{k}")
                nc.sync.dma_start(t[:], xT[k * L:(k + 1) * L, :])
                xt_sb.append(t)

            u_dt = work.tile([DP, T], f4, tag="u_dt")
            u_tp = work.tile([L, FC], f4, tag="u_tp")
            s_all = work.tile([NJ, FC], f4, tag="s_all")
            e_all = work.tile([NJ, FC], f4, tag="e_all")
            f_all = work.tile([NJ, FC], f4, tag="f_all")
            f2_all = work.tile([(NS - 1) * R, FC], f4, tag="f2_all")
            base_sb = work.tile([DP, T], f4, tag="base_sb")
            out_sb = work.tile([DP, T], f4, tag="out_sb")

            # ---- projection: u_dt[d, t] = sum_e mi[e, d] * xT[e, t]
            for n in range(NSPL):
                pu = psp.tile([DP, 512], f4, tag="ps")
                for k in range(KT):
                    nc.tensor.matmul(
                        pu[:], mi_sb[k], xt_sb[k][:, n * 512:(n + 1) * 512],
                        start=(k == 0), stop=(k == KT - 1))
                nc.scalar.copy(u_dt[:, n * 512:(n + 1) * 512], pu[:])

            # ---- base triangle (exact, lags 0..7) in d-partition layout.
            nc.scalar.activation(base_sb[:], u_dt[:],
                                 mybir.ActivationFunctionType.Copy,
                                 scale=ktab_sb[:, 0:1])
            for dlt in range(1, SUB):
                ov = base_sb[:].rearrange(
                    "d (sb l) -> d sb l", l=SUB)[:, :, dlt:SUB]
                uv = u_dt[:].rearrange(
                    "d (sb l) -> d sb l", l=SUB)[:, :, 0:SUB - dlt]
                nc.vector.scalar_tensor_tensor(
                    ov, uv, ktab_sb[:, dlt:dlt + 1], ov,
                    op0=mybir.AluOpType.mult, op1=mybir.AluOpType.add)

            # ---- u_tp: per-chunk transpose of u_dt, (d,c)-interleaved cols
            for c in range(NCH):
                ptp = psp.tile([L, DP], f4, tag="ps")
                nc.tensor.transpose(ptp[:], u_dt[:, c * L:(c + 1) * L], id_sb)
                nc.scalar.copy(u_tp[:, c::NCH], ptp[:])

            # ---- chunk states: 3 wide matmuls + scan
            spsum = bigp.tile([NJ, FC], f4, tag="big", name="spsum")
            for n in range(3):
                nc.tensor.matmul(spsum[:, n * 512:(n + 1) * 512], qt_sb,
                                 u_tp[:, n * 512:(n + 1) * 512],
                                 start=True, stop=True)
            nc.scalar.copy(s_all[:], spsum[:])
            nc.vector.tensor_tensor_scan(
                e_all[:], gate_sb, s_all[:], 0.0,
                op0=mybir.AluOpType.mult, op1=mybir.AluOpType.add)
            nc.vector.tensor_tensor(f_all[:], e_all[:], wrep_sb,
                                    op=mybir.AluOpType.mult)

            # ---- sub states: 3 wide matmuls + fold fitted weights
            epsum = bigp.tile([(NS - 1) * R, FC], f4, tag="big", name="epsum")
            for n in range(3):
                nc.tensor.matmul(epsum[:, n * 512:(n + 1) * 512], rt_sb,
                                 u_tp[:, n * 512:(n + 1) * 512],
                                 start=True, stop=True)
            nc.vector.tensor_tensor(f2_all[:], epsum[:], vrep_sb,
                                    op=mybir.AluOpType.mult)

            # ---- per chunk: carry matmuls into PSUM, then merge with base
            for c in range(NCH):
                sacc = cpool.tile([DP, L], f4, tag="sacc", bufs=3)
                clhs = zeros_sb if c == 0 else f_all[:, (c - 1)::NCH]
                nc.tensor.matmul(sacc[:], clhs, pt_sb, start=True, stop=False)
                nc.tensor.matmul(sacc[:, SUB:L], f2_all[:, c::NCH], p2_sb,
                                 start=False, stop=True)
                nc.vector.tensor_tensor(
                    out_sb[:, c * L:(c + 1) * L], sacc[:],
                    base_sb[:, c * L:(c + 1) * L], op=mybir.AluOpType.add)

            nc.sync.dma_start(out[:], out_sb[:])
    nc.compile()
    return nc


def _get_program():
    if "nc" not in _CACHE:
        _CACHE["nc"] = _build_nc()
    return _CACHE["nc"]


def kernel(x, input_pos, M_inputs, M_filters, A, Bm, C, Dv, _trace=False,
           _trace_kwargs=None):
    from concourse.bass_utils import run_bass_kernel_spmd

    x = np.asarray(x, dtype=np.float32)
    per_core = _derive_tables(
        np.asarray(A), np.asarray(Bm), np.asarray(C), np.asarray(Dv),
        np.asarray(M_filters), np.asarray(M_inputs))
    xTm = np.ascontiguousarray(x[0].T)                   # (768, 2048)

    nc = _get_program()
    in_maps = [dict(xT=xTm, **per_core[i]) for i in range(NCORE)]
    kw = dict(_trace_kwargs or {})
    res = run_bass_kernel_spmd(nc, in_maps, list(range(NCORE)),
                               trace=_trace, **kw)
    _CACHE["last_result"] = res
    full = np.concatenate([res.results[i]["out"] for i in range(NCORE)], axis=0)
    return np.ascontiguousarray(full.T)[None].astype(np.float32)


if __name__ == "__main__":
    rng = np.random.default_rng(0)
    ins = dict(
        x=rng.standard_normal((1, T, D), dtype=np.float32),
        input_pos=np.arange(T, dtype=np.int32),
        M_inputs=(rng.standard_normal((D, D)) * 0.02).astype(np.float32),
        M_filters=(rng.standard_normal((24, D)) * 0.02).astype(np.float32),
        A=rng.uniform(0, 0.99, 100).astype(np.float32),
        Bm=(rng.standard_normal(100) * 0.1).astype(np.float32),
        C=(rng.standard_normal((100, 48)) * 0.1).astype(np.float32),
        Dv=(rng.standard_normal(48) * 0.1).astype(np.float32),
    )
    got = kernel(**ins)
    print("kernel output", got.shape, got.dtype, float(np.abs(got).max()))


# revision 21
# speedup vs baseline: 1.5241x; 1.0352x over previous
"""Trainium2 Bass kernel for nn_DistillSTU (LDS scan + spectral contraction).

Math: out[t,d] = sum_{delta>=0} k[delta,d] * u[t-delta,d],  u = x @ M_inputs,
      k[delta,d] = sum_j W[j,d]*Bm[j]*A[j]^delta (+ dvg[d] at delta=0),
      W = (C[:,:24]+C[:,24:]) @ M_filters, dvg = (Dv[:24]+Dv[24:]) @ M_filters.

Sharding: 768 channels split across 8 cores (96 each); embarrassingly parallel.

Per-core decomposition over T=2048 (chunks L=128, subs l=8):
  base   same-sub pairs (lag 0..7): exact short kernel; shift-FMA on
         ScalarE (lag 0) + GpSimd/VectorE (lags 1..7), d-partition layout.
  sub    same-chunk earlier-sub pairs: reduced-pole (r=8) states, batched
         across all chunks into 3 wide matmuls; one carry matmul per chunk.
  chunk  earlier-chunk pairs: exact 100 poles; chunk states via 3 wide
         matmuls + one tensor_tensor_scan; one carry matmul per chunk.
All state tensors use the (d,c)-interleaved free layout (col = d*NCH + c)
so per-chunk slices are stride-NCH column views.
"""
import sys
import numpy as np

sys.path.insert(0, "/opt/trn_rl_repo")

T = 2048
D = 768
NJ = 100          # exact state dim (chunk path)
L = 128           # chunk length
NCH = T // L      # 16 chunks
SUB = 8           # sub length
NS = L // SUB     # 16 subs per chunk
R = 8             # reduced poles for sub-carries; (s,p) = 15*8 = 120 <= 128
NCORE = 8
DP = D // NCORE   # 96 channels per core
FC = DP * NCH     # 1536 free cols of the (d,c) layout

_CACHE = {}

# column offsets inside the packed constant blocks (partition dim = 128)
_CONST_WIDTHS = [
    ("mi", 6 * DP), ("qt", NJ), ("pt", L), ("rt", (NS - 1) * R),
    ("p2", (NS - 1) * SUB), ("ktab", SUB), ("ident", DP), ("zeros", DP),
]
_CONST2_WIDTHS = [("gate", FC), ("wrep", FC), ("vrep", FC)]
CONST_OFF = {}
_off = 0
for _n, _w in _CONST_WIDTHS:
    CONST_OFF[_n] = _off
    _off += _w
CW = _off
CONST2_OFF = {}
_off = 0
for _n, _w in _CONST2_WIDTHS:
    CONST2_OFF[_n] = _off
    _off += _w
CW2 = _off


def _derive_tables(A, Bm, C, Dv, M_filters, M_inputs):
    """All host-side parameter preprocessing (small tensors only)."""
    f8 = np.float64
    A = A.astype(f8); Bm = Bm.astype(f8)
    C = C.astype(f8); Dv = Dv.astype(f8); Mf = M_filters.astype(f8)
    W = (C[:, :24] + C[:, 24:]) @ Mf                    # (100, 768)
    dvg = (Dv[:24] + Dv[24:]) @ Mf                      # (768,)
    V100 = W * Bm[:, None]                              # (100, 768)

    # exact short kernel (lags 0..7)
    pows = A[None, :] ** np.arange(SUB)[:, None]        # (8, 100)
    ktab8 = pows @ V100                                 # (8, 768)
    ktab8[0] += dvg

    # reduced-pole fit of k[delta,d] on delta in [1, L-1]; pole decay
    # rates refined by Nelder-Mead on the least-squares residual
    deltas = np.arange(1, L)
    kwin = (A[None, :] ** deltas[:, None]) @ V100       # (127, 768)

    def _fit(lam):
        mu = np.exp(-np.abs(lam))
        G = mu[None, :] ** deltas[:, None]
        Vr, *_ = np.linalg.lstsq(G, kwin, rcond=None)
        return mu, G, Vr, np.linalg.norm(G @ Vr - kwin)

    lam = np.geomspace(0.02, 1.5, R)
    mu, G, Vr, r0 = _fit(lam)
    try:
        from scipy.optimize import minimize
        res = minimize(lambda x: _fit(x)[3], lam, method="Nelder-Mead",
                       options={"maxiter": 3000, "fatol": 1e-12})
        mu2, G2, Vr2, r2 = _fit(res.x)
        if r2 < r0:
            mu, G, Vr = mu2, G2, Vr2
    except Exception:
        pass

    # chunk-level tables (exact poles)
    qt = (Bm[None, :] * A[None, :] ** (L - 1 - np.arange(L))[:, None])  # (128,100)
    pt = np.ascontiguousarray(
        (A[None, :] ** (np.arange(L) + 1)[:, None]).T)  # (100,128): A_j^{i+1}
    gate = np.broadcast_to((A ** L)[:, None], (NJ, FC)).copy()
    gate[:, 0::NCH] = 0.0                               # reset at c==0 per channel

    # sub-level tables (reduced poles); (s,p) order: s=1..15 outer, p inner
    rt = np.zeros((L, (NS - 1) * R))
    for s in range(1, NS):
        m = np.arange(SUB * s)
        rt[: SUB * s, (s - 1) * R:s * R] = mu[None, :] ** (SUB * s - 1 - m)[:, None]
    p2 = np.zeros(((NS - 1) * R, (NS - 1) * SUB))       # block-diag carries
    pr = mu[:, None] ** (np.arange(SUB) + 1)[None, :]   # (R, 8)
    for s in range(NS - 1):
        p2[s * R:(s + 1) * R, s * SUB:(s + 1) * SUB] = pr

    f4 = np.float32
    per_core = []
    for i in range(NCORE):
        sl = slice(i * DP, (i + 1) * DP)
        wrep = np.repeat(W[:, sl][:, :, None], NCH, axis=2).reshape(NJ, FC)
        vrep = np.zeros(((NS - 1) * R, FC))
        vr_dc = np.repeat(Vr[:, sl][:, :, None], NCH, axis=2).reshape(R, FC)
        for s in range(NS - 1):
            vrep[s * R:(s + 1) * R] = vr_dc
        mi = np.ascontiguousarray(M_inputs.astype(f8)[:, sl]).astype(f4)
        ktabT = np.ascontiguousarray(ktab8[:, sl].T)    # (96, 8)
        cb = np.zeros((128, CW), dtype=f4)
        for name, arr in (
            ("mi", mi.reshape(6, 128, DP).transpose(1, 0, 2).reshape(128, 6 * DP)),
            ("qt", qt), ("pt", pt), ("rt", rt), ("p2", p2), ("ktab", ktabT),
            ("ident", np.eye(DP)), ("zeros", np.zeros((NJ, DP))),
        ):
            c0 = CONST_OFF[name]
            cb[:arr.shape[0], c0:c0 + arr.shape[1]] = arr
        cb2 = np.zeros((128, CW2), dtype=f4)
        for name, arr in (("gate", gate), ("wrep", wrep), ("vrep", vrep)):
            c0 = CONST2_OFF[name]
            cb2[:arr.shape[0], c0:c0 + arr.shape[1]] = arr
        per_core.append(dict(consts=cb, consts2=cb2))
    return per_core


def _build_nc():
    from concourse import bass, bacc, mybir, tile

    nc = bacc.Bacc()
    f4 = mybir.dt.float32
    xT = nc.declare_dram_parameter("xT", [D, T], f4, isOutput=False)
    cdram = nc.declare_dram_parameter("consts", [128, CW], f4, isOutput=False)
    cdram2 = nc.declare_dram_parameter("consts2", [128, CW2], f4, isOutput=False)
    out = nc.declare_dram_parameter("out", [DP, T], f4, isOutput=True)

    KT = D // L   # 6 k-tiles for the projection contraction
    NSPL = T // 512

    with tile.TileContext(nc) as tc:
        with (
            tc.tile_pool(name="consts", bufs=1) as consts,
            tc.tile_pool(name="xt", bufs=1) as xtp,
            tc.tile_pool(name="work", bufs=1) as work,
            tc.tile_pool(name="ps", bufs=2, space="PSUM") as psp,
            tc.tile_pool(name="big", bufs=3, space="PSUM") as bigp,
            tc.tile_pool(name="carry", bufs=3, space="PSUM") as cpool,
        ):
            call = consts.tile([128, CW], f4, tag="call")
            nc.sync.dma_start(call[:], cdram[:])
            call2 = consts.tile([128, CW2], f4, tag="call2")
            nc.sync.dma_start(call2[:], cdram2[:])

            def cs(name, rows, width, woff=0):
                c0 = CONST_OFF[name] + woff
                return call[0:rows, c0:c0 + width]

            def cs2(name, rows, width):
                c0 = CONST2_OFF[name]
                return call2[0:rows, c0:c0 + width]

            mi_sb = [cs("mi", 128, DP, k * DP) for k in range(KT)]
            qt_sb = cs("qt", L, NJ)
            pt_sb = cs("pt", NJ, L)
            gate_sb = cs2("gate", NJ, FC)
            wrep_sb = cs2("wrep", NJ, FC)
            rt_sb = cs("rt", L, (NS - 1) * R)
            p2_sb = cs("p2", (NS - 1) * R, (NS - 1) * SUB)
            vrep_sb = cs2("vrep", (NS - 1) * R, FC)
            ktab_sb = cs("ktab", DP, SUB)
            id_sb = cs("ident", DP, DP)
            zeros_sb = cs("zeros", NJ, DP)

            xt_sb = []
            for k in range(KT):
                t = xtp.tile([L, T], f4, tag=f"xt{k}", name=f"xt_sb{k}")
                nc.sync.dma_start(t[:], xT[k * L:(k + 1) * L, :])
                xt_sb.append(t)

            u_dt = work.tile([DP, T], f4, tag="u_dt")
            u_tp = work.tile([L, FC], f4, tag="u_tp")
            s_all = work.tile([NJ, FC], f4, tag="s_all")
            e_all = work.tile([NJ, FC], f4, tag="e_all")
            f_all = work.tile([NJ, FC], f4, tag="f_all")
            f2_all = work.tile([(NS - 1) * R, FC], f4, tag="f2_all")
            base_sb = work.tile([DP, T], f4, tag="base_sb")
            out_sb = work.tile([DP, T], f4, tag="out_sb")

            # ---- projection: u_dt[d, t] = sum_e mi[e, d] * xT[e, t]
            for n in range(NSPL):
                pu = psp.tile([DP, 512], f4, tag="ps")
                for k in range(KT):
                    nc.tensor.matmul(
                        pu[:], mi_sb[k], xt_sb[k][:, n * 512:(n + 1) * 512],
                        start=(k == 0), stop=(k == KT - 1))
                nc.scalar.copy(u_dt[:, n * 512:(n + 1) * 512], pu[:])

            # ---- base triangle (exact, lags 0..7) in d-partition layout.
            nc.scalar.activation(base_sb[:], u_dt[:],
                                 mybir.ActivationFunctionType.Copy,
                                 scale=ktab_sb[:, 0:1])
            for dlt in range(1, SUB):
                ov = base_sb[:].rearrange(
                    "d (sb l) -> d sb l", l=SUB)[:, :, dlt:SUB]
                uv = u_dt[:].rearrange(
                    "d (sb l) -> d sb l", l=SUB)[:, :, 0:SUB - dlt]
                nc.vector.scalar_tensor_tensor(
                    ov, uv, ktab_sb[:, dlt:dlt + 1], ov,
                    op0=mybir.AluOpType.mult, op1=mybir.AluOpType.add)

            # ---- u_tp: per-chunk transpose of u_dt, (d,c)-interleaved cols
            for c in range(NCH):
                ptp = psp.tile([L, DP], f4, tag="ps")
                nc.tensor.transpose(ptp[:], u_dt[:, c * L:(c + 1) * L], id_sb)
                nc.scalar.copy(u_tp[:, c::NCH], ptp[:])

            # ---- chunk states: 3 bank-sized matmuls + scan
            for n in range(3):
                sp = bigp.tile([NJ, 512], f4, tag="big", name=f"sp{n}")
                nc.tensor.matmul(sp[:], qt_sb,
                                 u_tp[:, n * 512:(n + 1) * 512],
                                 start=True, stop=True)
                nc.scalar.copy(s_all[:, n * 512:(n + 1) * 512], sp[:])
            nc.vector.tensor_tensor_scan(
                e_all[:], gate_sb, s_all[:], 0.0,
                op0=mybir.AluOpType.mult, op1=mybir.AluOpType.add)
            nc.vector.tensor_tensor(f_all[:], e_all[:], wrep_sb,
                                    op=mybir.AluOpType.mult)

            # ---- sub states: 3 bank-sized matmuls + fold fitted weights
            for n in range(3):
                ep = bigp.tile([(NS - 1) * R, 512], f4, tag="big", name=f"ep{n}")
                nc.tensor.matmul(ep[:], rt_sb,
                                 u_tp[:, n * 512:(n + 1) * 512],
                                 start=True, stop=True)
                nc.vector.tensor_tensor(
                    f2_all[:, n * 512:(n + 1) * 512], ep[:],
                    vrep_sb[:, n * 512:(n + 1) * 512],
                    op=mybir.AluOpType.mult)

            # ---- per chunk: carry matmuls into PSUM, then merge with base
            for c in range(NCH):
                sacc = cpool.tile([DP, L], f4, tag="sacc", bufs=3)
                clhs = zeros_sb if c == 0 else f_all[:, (c - 1)::NCH]
                nc.tensor.matmul(sacc[:], clhs, pt_sb, start=True, stop=False)
                nc.tensor.matmul(sacc[:, SUB:L], f2_all[:, c::NCH], p2_sb,
                                 start=False, stop=True)
                nc.vector.tensor_tensor(
                    out_sb[:, c * L:(c + 1) * L], sacc[:],
                    base_sb[:, c * L:(c + 1) * L], op=mybir.AluOpType.add)

            nc.sync.dma_start(out[:], out_sb[:])
    nc.compile()
    return nc


def _get_program():
    if "nc" not in _CACHE:
        _CACHE["nc"] = _build_nc()
    return _CACHE["nc"]


def kernel(x, input_pos, M_inputs, M_filters, A, Bm, C, Dv, _trace=False,
           _trace_kwargs=None):
    from concourse.bass_utils import run_bass_kernel_spmd

    x = np.asarray(x, dtype=np.float32)
    per_core = _derive_tables(
        np.asarray(A), np.asarray(Bm), np.asarray(C), np.asarray(Dv),
        np.asarray(M_filters), np.asarray(M_inputs))
    xTm = np.ascontiguousarray(x[0].T)                   # (768, 2048)

    nc = _get_program()
    in_maps = [dict(xT=xTm, **per_core[i]) for i in range(NCORE)]
    kw = dict(_trace_kwargs or {})
    res = run_bass_kernel_spmd(nc, in_maps, list(range(NCORE)),
                               trace=_trace, **kw)
    _CACHE["last_result"] = res
    full = np.concatenate([res.results[i]["out"] for i in range(NCORE)], axis=0)
    return np.ascontiguousarray(full.T)[None].astype(np.float32)


if __name__ == "__main__":
    rng = np.random.default_rng(0)
    ins = dict(
        x=rng.standard_normal((1, T, D), dtype=np.float32),
        input_pos=np.arange(T, dtype=np.int32),
        M_inputs=(rng.standard_normal((D, D)) * 0.02).astype(np.float32),
        M_filters=(rng.standard_normal((24, D)) * 0.02).astype(np.float32),
        A=rng.uniform(0, 0.99, 100).astype(np.float32),
        Bm=(rng.standard_normal(100) * 0.1).astype(np.float32),
        C=(rng.standard_normal((100, 48)) * 0.1).astype(np.float32),
        Dv=(rng.standard_normal(48) * 0.1).astype(np.float32),
    )
    got = kernel(**ins)
    print("kernel output", got.shape, got.dtype, float(np.abs(got).max()))
